# revision 38
# baseline (speedup 1.0000x reference)
"""Trainium2 Bass kernel for nn_ASSANetEncoder (point-cloud set-abstraction encoder).

Reference computation per batch b (B=8, N=16384, P=4096, S=32, C_in=64, C=128):
    neigh[c,p,s] = support_features[c, idx[p,s]]
    rel[d,p,s]   = support_xyz[idx[p,s], d] - query_xyz[p, d]
    agg[c,p,s]   = neigh[c,p,s] * rel[d(c),p,s]      (d(c): 21/21/22 repeat-interleave)
    y1 = relu(scale1*(W1@agg) + shift1)              (inference BN folded to scale/shift)
    y2 = scale2*(W2@y1) + shift2
    out[o,p]     = max_s relu(y2)

Sharding: data-parallel over batch, one batch per NeuronCore (8 cores).

Per-core kernel strategy (v2 — non-transpose gather + XBAR transpose):
  * A per-batch fp16 table with rows [f*g_rep (64ch) | f (64ch)] lives in HBM
    ([N, 128] rows, 256 B each).
  * agg = f*(g_rep - q_rep) is decomposed as f*g_rep - f*q_rep:
      - the f*g_rep product is precomputed per support point (row top half)
      - f*q_rep is one fp16 tensor_tensor multiply on the gathered rows
      - the subtraction folds into the first matmul by stacking [W1'; -W1']
        as a K=128 lhsT.  BN scales fold into the conv weights; shifts ride
        the ReLU activations as per-partition bias; max over S commutes with
        the final (bias+relu).
  * Gathers use dma_gather(transpose=False): each index is one contiguous
    256 B HBM->SBUF copy landing whole in one partition (sample-major
    layout), rotated across 4 SWDGE queues.  The 4-queue rotation roughly
    halves the SWDGE serial cost vs a single queue (ring-drain stalls
    overlap across queue pairs).  Transpose-mode gathers push data through
    the SBUF XBAR, which corrupts when gathers on different queues overlap
    — transpose=False is XBAR-free, so multi-queue is safe.
  * Tile assigns Pool-engine DMA insts round-robin to 8 DMASW sem lanes
    (lane = pool-DMA inst index % 8).  A lane's counting sem only orders
    correctly if all its insts share one HW queue, so queue_num follows the
    same instruction counter mod NQUEUES (4 divides 8).
  * The sample-major gather block [128, K, 128] is flipped to channel-major
    on the Tensor engine: 4 is_transpose matmuls per 512-sample chunk land
    [128, 4, 128] f32 in one PSUM bank; one scalar Copy turns that into the
    fp16 channel-major rhs.  (An XBAR dma_start_transpose would be cheaper
    on paper, but its transfers stall the SDMA engines that drain the
    gather rings — measured +25 us per slice — so the PE does it instead.)
  * The f * q_rep multiply happens pre-transpose in sample-major layout
    ([128 part, K, 64] vs [64, K*128] post-transpose: all partitions busy,
    half the DVE time), against a host-precomputed per-slice q_rep tile.
  * Matmuls consume 512-column chunks (one PSUM bank).  First/last slices
    are split into sub-gathers so the pipeline ramps as soon as an 8 KB idx
    head lands and drains incrementally at the tail.
"""

import os
import sys

sys.path.insert(0, "/opt/trn_rl_repo")

import numpy as np

B, N, NPOINT, NSAMPLE = 8, 16384, 4096, 32
C_IN, C_MID, C_OUT = 64, 128, 128
EPS = 1e-5
REPEATS = [21, 21, 22]

CHUNK = 512                    # matmul free dim / PSUM bank
GIDX = int(os.environ.get("K_GIDX", str(NPOINT)))  # indices per slice
NQUEUES = int(os.environ.get("K_QUEUES", "4"))     # SWDGE queue rotation
NBUF_NT = int(os.environ.get("K_NBUF_NT", "8"))
NBUF_T = int(os.environ.get("K_NBUF_T", "4"))
# SWDGE descriptor carveout bytes/partition: each queue's per-engine ring
# holds SCRATCH/16 descriptors; a 4096-idx gather needs 257 per engine.
SCRATCH = int(os.environ.get("K_SCRATCH", "16384"))
# HW queue id per rotation step (fixed function of the pool-DMA counter
# mod NQUEUES, so Tile's DMASW lanes stay single-queue)
QPERM = [int(x) for x in os.environ.get("K_QPERM", "0,2,1,3").split(",")]

_compiled = None


def _build():
    import concourse.tile as tile
    from concourse import bacc, mybir

    f16 = mybir.dt.float16
    f32 = mybir.dt.float32
    i16 = mybir.dt.int16
    Alu = mybir.AluOpType
    Act = mybir.ActivationFunctionType

    nc = bacc.Bacc("TRN2", target_bir_lowering=False, debug=False,
                   enable_asserts=False, num_devices=8,
                   num_swdge_queues=NQUEUES,
                   dynamic_dma_scratch_size=SCRATCH)

    table_d = nc.dram_tensor("table", [N, 128], f16, kind="ExternalInput")
    idx_d = nc.dram_tensor("idx", [128, NPOINT * NSAMPLE // 16], i16,
                           kind="ExternalInput")
    w_d = nc.dram_tensor("wstack", [128, 256], f16, kind="ExternalInput")
    c_d = nc.dram_tensor("consts", [128, 2], f32, kind="ExternalInput")
    id_d = nc.dram_tensor("ident", [128, 128], f16, kind="ExternalInput")
    out_d = nc.dram_tensor("out", [C_OUT, NPOINT], f32, kind="ExternalOutput")

    NG = NPOINT * NSAMPLE // GIDX
    SPG = max(1, GIDX // NPOINT)          # s-slices per gather
    BLK = GIDX // 128                     # 128-sample blocks per gather
    # sample-major q_rep tile: qnt[q, a, e] = q_rep[64+e, a*128+q]
    qn_d = nc.dram_tensor("qnt", [128, SPG * NPOINT // 128, C_IN], f16,
                          kind="ExternalInput")

    with tile.TileContext(nc) as tc:
        with (
            tc.tile_pool(name="const", bufs=1) as cpool,
            tc.tile_pool(name="gnt", bufs=NBUF_NT) as ntpool,
            tc.tile_pool(name="gt", bufs=NBUF_T) as tpool,
            tc.tile_pool(name="y1r", bufs=4) as rpool,
            tc.tile_pool(name="pst", bufs=2, space="PSUM") as pst,
            tc.tile_pool(name="ps1", bufs=2, space="PSUM") as ps1,
            tc.tile_pool(name="ps2", bufs=4, space="PSUM") as ps2,
        ):
            idx = cpool.tile([128, NPOINT * NSAMPLE // 16], i16, tag="idx")
            # first gather's head lands first so the pipeline starts as soon
            # as the DMA queues come up (Tile tracks per-DMA ranges)
            nc.sync.dma_start(idx[:, :32], idx_d.ap()[:, :32])
            nc.sync.dma_start(idx[:, 32:GIDX // 16], idx_d.ap()[:, 32:GIDX // 16])
            w = cpool.tile([128, 256], f16, tag="w")
            nc.scalar.dma_start(w[:], w_d.ap()[:])
            consts = cpool.tile([128, 2], f32, tag="consts")
            nc.scalar.dma_start(consts[:], c_d.ap()[:])
            ident = cpool.tile([128, 128], f16, tag="ident")
            nc.scalar.dma_start(ident[:], id_d.ap()[:])
            qnt = cpool.tile([128, SPG * NPOINT // 128, C_IN], f16, tag="qnt")
            nc.scalar.dma_start(qnt[:], qn_d.ap()[:])
            # progressive tail load: gather g only waits for its own range
            lo = GIDX // 16
            tot_cols = NPOINT * NSAMPLE // 16
            step = GIDX // 16
            while lo < tot_cols:
                hi = min(tot_cols, lo + step)
                nc.sync.dma_start(idx[:, lo:hi], idx_d.ap()[:, lo:hi])
                lo = hi
                step *= 2
            acc = cpool.tile([128, NPOINT], f32, tag="acc")
            outt = cpool.tile([128, NPOINT], f32, tag="outt")

            # Tile DMASW-lane alignment: queue_num = (pool-DMA inst idx) % NQUEUES
            dma_ctr = 0
            nregs = {}

            def get_nreg(n):
                if n not in nregs:
                    nregs[n] = nc.gpsimd.to_reg(n)
                return nregs[n]

            get_nreg(GIDX)

            for g in range(NG):
                # gather parts: ramp up the first gather, drain the last
                if g == 0:
                    parts = [(0, 512), (512, GIDX)]
                elif g < NG - 1:
                    parts = [(0, GIDX)]
                else:
                    q = GIDX // 4
                    parts = [(0, q), (q, 2 * q), (2 * q, 3 * q),
                             (3 * q, GIDX)]
                G = ntpool.tile([128, BLK, 128], f16, tag="G")
                for (lo, hi) in parts:
                    n = hi - lo
                    a0, a1 = lo // 128, hi // 128
                    nc.gpsimd.dma_gather(
                        G[:, a0:a1, :],
                        table_d.ap()[:],
                        idx[:, (g * GIDX + lo) // 16:(g * GIDX + hi) // 16],
                        n,
                        get_nreg(n),
                        128,
                        transpose=False,
                        single_packet=False,
                        queue_num=QPERM[dma_ctr % NQUEUES],
                    )
                    dma_ctr += 1
                    # bottom half: f * q_rep in place, sample-major
                    qb = ((g * GIDX) % NPOINT) // 128
                    nc.vector.tensor_tensor(
                        G[:, a0:a1, 64:128], G[:, a0:a1, 64:128],
                        qnt[:, qb + a0:qb + a1, :], Alu.mult)
                def emit_transpose_copy(pr):
                    # PE block transposes for a PAIR of chunks -> one full
                    # PSUM bank [128, 8, 128] f16, then ONE scalar Copy of
                    # [128, 1024] to the fp16 matmul rhs (halves the scalar
                    # copy op count and its sem round trips)
                    blk0 = pr * 2 * (CHUNK // 128)
                    pt = pst.tile([128, 2 * CHUNK // 128, 128], f16, tag="pt")
                    for i in range(2 * CHUNK // 128):
                        nc.tensor.transpose(pt[:, i, :], G[:, blk0 + i, :],
                                            ident[:])
                    gt = tpool.tile([128, 2 * CHUNK], f16, tag="gt")
                    nc.scalar.activation(gt[:],
                                         pt.rearrange("p a n -> p (a n)"),
                                         Act.Copy)
                    return gt

                # software pipeline: the next pair's transpose+copy is
                # emitted BEFORE this pair's matmul/relu stage, so the
                # in-order scalar queue never parks a ready Copy behind a
                # Relu that is waiting on a PE round trip
                NPAIR = GIDX // CHUNK // 2
                gt_next = emit_transpose_copy(0)
                for cc in range(GIDX // CHUNK):
                    if cc % 2 == 0:
                        gtp = gt_next
                        if cc // 2 + 1 < NPAIR:
                            gt_next = emit_transpose_copy(cc // 2 + 1)
                    gt = gtp[:, (cc % 2) * CHUNK:(cc % 2 + 1) * CHUNK]
                    pos = g * GIDX + cc * CHUNK
                    s = pos // NPOINT
                    acs = slice(pos % NPOINT, pos % NPOINT + CHUNK)
                    y1 = ps1.tile([128, CHUNK], f32, tag="y1")
                    nc.tensor.matmul(y1[:], w[:, 0:128], gt,
                                     start=True, stop=True)
                    y1r = rpool.tile([128, CHUNK], f16, tag="y1r")
                    nc.scalar.activation(y1r[:], y1[:], Act.Relu,
                                         bias=consts[:, 0:1], scale=1.0)
                    y2 = ps2.tile([128, CHUNK], f32, tag="y2")
                    nc.tensor.matmul(y2[:], w[:, 128:256], y1r[:],
                                     start=True, stop=True)
                    if s == 0:
                        nc.scalar.activation(acc[:, acs], y2[:], Act.Copy)
                    else:
                        nc.vector.tensor_tensor(acc[:, acs], y2[:], acc[:, acs],
                                                Alu.max)
                    if s == NSAMPLE - 1:
                        nc.scalar.activation(outt[:, acs], acc[:, acs], Act.Relu,
                                             bias=consts[:, 1:2], scale=1.0)
                        if cc == GIDX // CHUNK - 1:
                            # final chunk: split the writeback across two
                            # queues to shorten the drain
                            h = slice(acs.start, acs.start + CHUNK // 2)
                            h2 = slice(acs.start + CHUNK // 2, acs.stop)
                            nc.sync.dma_start(out_d.ap()[:, h], outt[:, h])
                            nc.scalar.dma_start(out_d.ap()[:, h2], outt[:, h2])
                        else:
                            nc.sync.dma_start(out_d.ap()[:, acs], outt[:, acs])

    nc.compile()
    return nc


def _get_compiled():
    global _compiled
    if _compiled is None:
        _compiled = _build()
    return _compiled


_IDENT = np.ascontiguousarray(np.eye(128, dtype=np.float16))


def _prep_core_inputs(b, query_xyz, support_xyz, support_features, neighbor_idx,
                      wstack, consts):
    f = np.asarray(support_features[b], np.float32)            # [64, N]
    grep = np.repeat(np.asarray(support_xyz[b], np.float32).T,
                     REPEATS, axis=0)                          # [64, N]
    table = np.ascontiguousarray(
        np.concatenate([(f * grep).T, f.T], axis=1).astype(np.float16))

    stream = np.asarray(neighbor_idx[b], np.int64).T.reshape(-1)  # [S*P], p fastest
    wrapped = stream.astype(np.int16).reshape(-1, 16).T           # [16, S*P/16]
    idx = np.ascontiguousarray(np.tile(wrapped, (8, 1)))          # [128, S*P/16]

    qirep = np.repeat(np.asarray(query_xyz[b], np.float32).T,
                      REPEATS, axis=0).astype(np.float16)         # [64, P]
    # sample-major layout: qnt[q, a, e] = qirep[e, a*128 + q]
    spg = max(1, GIDX // NPOINT)
    qnt_one = np.ascontiguousarray(
        qirep.T.reshape(NPOINT // 128, 128, C_IN).transpose(1, 0, 2))
    qnt = np.ascontiguousarray(
        np.tile(qnt_one, (1, spg, 1)))            # [128, SPG*P/128, 64]

    return {"table": table, "idx": idx, "qnt": qnt,
            "wstack": wstack, "consts": consts, "ident": _IDENT}


def _ensure_trace_shim():
    """If BASS_TRACE is set but this image lacks antenv.axon_hooks, install a
    working shim (or a no-op) so run_bass_kernel_spmd never crashes."""
    try:
        import antenv.axon_hooks  # noqa: F401
        return
    except ImportError:
        pass
    import types
    import antenv
    hook = None
    try:
        from trn_agent_boot import trn_boot
        hook = trn_boot._ntff_profile_via_ctypes("/opt/axon/libaxon_pjrt.so")
    except Exception:
        hook = None
    shim = types.ModuleType("antenv.axon_hooks")
    shim.get_axon_ntff_profile_hook = lambda: hook
    shim.set_axon_ntff_profile_hook = lambda h: None
    sys.modules["antenv.axon_hooks"] = shim
    antenv.axon_hooks = shim


def kernel(query_xyz, support_xyz, support_features, neighbor_idx,
           W1, g1, b1, m1, v1, W2, g2, b2, m2, v2):
    from concourse.bass_utils import run_bass_kernel_spmd

    _ensure_trace_shim()

    nc = _get_compiled()

    scale1 = np.asarray(g1, np.float32) / np.sqrt(np.asarray(v1, np.float32) + EPS)
    shift1 = np.asarray(b1, np.float32) - np.asarray(m1, np.float32) * scale1
    scale2 = np.asarray(g2, np.float32) / np.sqrt(np.asarray(v2, np.float32) + EPS)
    shift2 = np.asarray(b2, np.float32) - np.asarray(m2, np.float32) * scale2

    W1p = (scale1[:, None] * np.asarray(W1, np.float32)).T     # [64, 128] lhsT
    W2p = (scale2[:, None] * np.asarray(W2, np.float32)).T     # [128, 128] lhsT
    lhsT1 = np.concatenate([W1p, -W1p], axis=0)                # [128, 128]
    wstack = np.ascontiguousarray(
        np.concatenate([lhsT1, W2p], axis=1)).astype(np.float16)
    consts = np.ascontiguousarray(np.stack([shift1, shift2], axis=1),
                                  dtype=np.float32)

    in_maps = [
        _prep_core_inputs(b, query_xyz, support_xyz, support_features,
                          neighbor_idx, wstack, consts)
        for b in range(B)
    ]

    res = run_bass_kernel_spmd(nc, in_maps, core_ids=list(range(B)))
    out = np.stack([res.results[b]["out"] for b in range(B)], axis=0)
    kernel.last_results = res
    return out.astype(np.float32)
